# revision 2
# baseline (speedup 1.0000x reference)
"""Multi-head attention on 8 Trainium2 NeuronCores, data-parallel over batch.

v2: x is transposed HOST-side (xt = x.T fed as input), eliminating the PE
transpose phase. All GEMMs bf16 N=512 matmuls, fp32 PSUM. Per core:
  qT/kT = Wqk.T @ xT   (feature-major, per head-pair)
  V     = xT.T @ Wv    (seq-major, into vaug with ones cols appended)
  per pair p, per s1-bank n (512 cols):
    scoresT[s2,s1] for heads 2p,2p+1 into a dual-head PSUM tile
      [128, 2, 512] via two row-tiled (K=64) concurrent matmuls
    exp: one ACT instr per m-chunk over [128, 2, 512] (1024 elem/lane)
    PV:  po[66,512] += vaug[m][:,h,:].T @ exp  (ones col -> row 64 = rowsum)
    normalize: outT[p] = po[0:64] * bcast(recip(po[64]))
  out = outT.T @ W_out + b_out  (first s1-bank emitted as PE filler
  during pair 7, second bank at the tail)
Projection matmuls for pair p+1 are interleaved as PE filler inside pair
p's score/PV loops so the PE never idles while ACT streams exps.
"""

import itertools

import ml_dtypes
import numpy as np

import concourse.bacc as bacc
import concourse.bass as bass
import concourse.mybir as mybir
from concourse.bass_utils import run_bass_kernel_spmd
from concourse.tile import TileContext
from concourse.tile_rust import add_dep_helper

F32 = mybir.dt.float32
BF16 = mybir.dt.bfloat16
AF = mybir.ActivationFunctionType

S = 1024       # sequence length
E = 1024       # embed dim
H = 16         # heads
D = 64         # head dim
P = 128        # partitions
NP = 8         # head pairs
KT = E // P    # contraction tiles (8)
SM = S // P    # seq tiles of 128 (8)
NB = S // 512  # seq banks of 512 (2)
SCALE = 1.0 / np.sqrt(D)


def build_nc():
    nc = bacc.Bacc(trn_type="TRN2", target_bir_lowering=False)
    xt = nc.dram_tensor("xt", [E, S], BF16, kind="ExternalInput")
    wqk = nc.dram_tensor("wqk", [E, 2 * E], BF16, kind="ExternalInput")
    wv = nc.dram_tensor("wv", [E, E], BF16, kind="ExternalInput")
    bqk = nc.dram_tensor("bqk", [2 * E], F32, kind="ExternalInput")
    bv = nc.dram_tensor("bv", [E], F32, kind="ExternalInput")
    wout = nc.dram_tensor("wout", [E, E], BF16, kind="ExternalInput")
    bout = nc.dram_tensor("bout", [E], F32, kind="ExternalInput")
    out = nc.dram_tensor("out", [S, E], F32, kind="ExternalOutput")

    with TileContext(nc) as tc:
        with (
            tc.tile_pool(name="const", bufs=1) as constp,
            tc.tile_pool(name="persist", bufs=1) as pers,
            tc.tile_pool(name="psum", bufs=1, space="PSUM") as psp,
            tc.tile_pool(name="wpool", bufs=1) as wp,
            tc.tile_pool(name="work", bufs=1) as wk,
        ):
            # ---- constants ----
            ones = constp.tile([1, 512], F32, tag="ones")
            nc.vector.memset(ones[:], 1.0)
            onespp = constp.tile([P, 2 * H], F32, tag="onespp")
            nc.vector.memset(onespp[:], 1.0)
            bcols = constp.tile([P, 2 * NP], F32, tag="bcols")
            nc.scalar.dma_start(bcols[:], bqk.ap().rearrange("(f p) -> p f", p=P))
            bvr = constp.tile([1, E], F32, tag="bvr")
            nc.scalar.dma_start(bvr[:], bv.ap()[None, :])
            botr = constp.tile([1, E], F32, tag="botr")
            nc.scalar.dma_start(botr[:], bout.ap()[None, :])

            # ---- persistent arrays ----
            xT = [pers.tile([P, S], BF16, tag=f"xt{k}", name=f"xT{k}")
                  for k in range(KT)]
            for k in range(KT):
                nc.sync.dma_start(xT[k][:], xt.ap()[bass.ts(k, P), :])
            vaug = [pers.tile([P, H, D + 2], BF16, tag=f"va{m}", name=f"vaug{m}")
                    for m in range(SM)]
            outT = [pers.tile([P, S], BF16, tag=f"ot{p}", name=f"outT{p}")
                    for p in range(NP)]
            bvb = constp.tile([P, E], F32, tag="bvb")
            boutb = constp.tile([P, E], F32, tag="boutb")

            # ones columns of vaug (independent of V eviction)
            for m in range(SM):
                nc.vector.tensor_copy(
                    vaug[m][:, :, D:D + 2],
                    onespp[:].rearrange("p (h t) -> p h t", h=H))

            # weight loads
            def load_wq(p):
                wq = []
                for k in range(KT):
                    w = wp.tile([P, 256], BF16, tag="wqk", bufs=16, name="wqk")
                    nc.sync.dma_start(
                        w[:], wqk.ap()[bass.ts(k, P), bass.ts(p, 256)])
                    wq.append(w)
                return wq

            def load_wv():
                wvk = [[], []]
                for n in range(NB):
                    for k in range(KT):
                        w = wp.tile([P, 512], BF16, tag="wv", bufs=16, name="wvk")
                        nc.sync.dma_start(
                            w[:], wv.ap()[bass.ts(k, P), bass.ts(n, 512)])
                        wvk[n].append(w)
                return wvk

            def load_wot(n):
                wot = []
                for k in range(KT):
                    w = wp.tile([P, 512], BF16, tag=f"wo{n}", bufs=8, name="wot")
                    nc.sync.dma_start(
                        w[:], wout.ap()[bass.ts(k, P), bass.ts(n, 512)])
                    wot.append(w)
                return wot

            wq_all = {0: load_wq(0)}
            wvk = load_wv()
            wq_all[1] = load_wq(1)

            # bias broadcast tiles via 1-row matmul trick
            for n in range(NB):
                cs = bass.ts(n, 512)
                pb = psp.tile([P, 512], F32, tag="mm", bufs=2, name="pb")
                nc.tensor.matmul(pb[:], ones[0:1, 0:P], bvr[0:1, cs])
                nc.vector.tensor_copy(bvb[:, cs], pb[:])
                pb2 = psp.tile([P, 512], F32, tag="mm", bufs=2, name="pb2")
                nc.tensor.matmul(pb2[:], ones[0:1, 0:P], botr[0:1, cs])
                nc.vector.tensor_copy(boutb[:, cs], pb2[:])

            # ---- projection generators (PE filler units) ----
            def alloc_qkt():
                qt = wk.tile([P, S], BF16, tag="qt", bufs=2, name="qt")
                kt = wk.tile([P, S], BF16, tag="kt", bufs=2, name="kt")
                return qt, kt

            def gen_qkproj(p, wq, qt, kt):
                """Yields once per matmul; projects q/k for pair p."""
                for which in range(2):
                    ws = slice(which * P, (which + 1) * P)
                    dst = qt if which == 0 else kt
                    bc = bcols[:, 2 * p + which:2 * p + which + 1]
                    for n in range(NB):
                        cs = bass.ts(n, 512)
                        ps = psp.tile([P, 512], F32, tag="mm", bufs=2,
                                      name="pproj")
                        for k in range(KT):
                            nc.tensor.matmul(
                                ps[:], wq[k][:, ws], xT[k][:, cs],
                                start=(k == 0), stop=(k == KT - 1))
                            yield
                        nc.vector.tensor_scalar_add(dst[:, cs], ps[:], bc)

            def gen_vproj(n):
                """Yields once per matmul; computes V bank n into vaug."""
                cs = bass.ts(n, 512)
                for m in range(SM):
                    pv = psp.tile([P, 512], F32, tag="mm", bufs=2, name="pvps")
                    for k in range(KT):
                        nc.tensor.matmul(
                            pv[:], xT[k][:, bass.ts(m, P)], wvk[n][k][:],
                            start=(k == 0), stop=(k == KT - 1))
                        yield
                    nc.vector.tensor_add(
                        vaug[m][:, bass.ts(n, 8), 0:D],
                        pv[:].rearrange("p (h d) -> p h d", h=8),
                        bvb[:, cs].rearrange("p (h d) -> p h d", h=8))

            def gen_outproj(n, wot, ms):
                cs = bass.ts(n, 512)
                for m in ms:
                    pf = psp.tile([P, 512], F32, tag="mm", bufs=2, name="pf")
                    for k in range(KT):
                        nc.tensor.matmul(
                            pf[:], outT[k][:, bass.ts(m, P)], wot[k][:],
                            start=(k == 0), stop=(k == KT - 1))
                        yield
                    osb = wk.tile([P, 512], F32, tag="osb", bufs=3, name="osb")
                    nc.vector.tensor_add(osb[:], pf[:], boutb[:, cs])
                    nc.sync.dma_start(out.ap()[bass.ts(m, P), cs], osb[:])

            def noop():
                while True:
                    yield

            def drain(gen, k):
                for _ in itertools.islice(gen, k):
                    pass

            def drain_all(gen):
                for _ in gen:
                    pass

            def chain(*gens):
                for g in gens:
                    yield from g

            # ---- phase 1: direct projections (startup) ----
            qt0, kt0 = alloc_qkt()
            drain_all(gen_qkproj(0, wq_all[0], qt0, kt0))
            drain_all(gen_vproj(0))
            qt1, kt1 = alloc_qkt()
            drain_all(gen_qkproj(1, wq_all[1], qt1, kt1))
            qkt = {0: (qt0, kt0), 1: (qt1, kt1)}

            # ---- phase 2: attention, pair-pipelined ----
            wot_all = {}
            for p in range(NP):
                # filler for this pair
                if p == 0:
                    filler = gen_vproj(1)
                elif p + 1 < NP:
                    wq_all[p + 1] = load_wq(p + 1)
                    qtn, ktn = alloc_qkt()
                    qkt[p + 1] = (qtn, ktn)
                    filler = gen_qkproj(p + 1, wq_all[p + 1], qtn, ktn)
                else:
                    filler = noop()
                if p == NP - 2:
                    wot_all[0] = load_wot(0)
                    wot_all[1] = load_wot(1)

                qt, kt = qkt[p]
                for n in range(NB):
                    cs = bass.ts(n, 512)
                    expt = wk.tile([P, SM, 2, 512], BF16, tag="exp", bufs=2,
                                   name="expt")
                    prev = None
                    for m in range(SM):
                        sc = psp.tile([P, 2, 512], F32, tag="sc", bufs=2,
                                      name="sc")
                        ia = nc.tensor.matmul(
                            sc[:, 0], kt[0:D, bass.ts(m, P)], qt[0:D, cs])
                        ib = nc.tensor.matmul(
                            sc[:, 1], kt[D:P, bass.ts(m, P)], qt[D:P, cs])
                        if prev is not None:
                            add_dep_helper(ia.ins, prev.ins, sync=False,
                                           reason="pair scores order")
                        add_dep_helper(ib.ins, ia.ins, sync=False,
                                       reason="pair scores order")
                        prev = ib
                        nc.scalar.activation(
                            expt[:, m], sc[:], AF.Exp, scale=SCALE)
                        drain(filler, 3)

                    poA = psp.tile([D + 2, 512], F32, tag="po", bufs=2,
                                   name="poA")
                    poB = psp.tile([D + 2, 512], F32, tag="po", bufs=2,
                                   name="poB")
                    for m in range(SM):
                        nc.tensor.matmul(
                            poA[:], vaug[m][:, 2 * p, :], expt[:, m, 0],
                            start=(m == 0), stop=(m == SM - 1))
                        nc.tensor.matmul(
                            poB[:], vaug[m][:, 2 * p + 1, :], expt[:, m, 1],
                            start=(m == 0), stop=(m == SM - 1))
                        drain(filler, 1)
                    for j, po in ((0, poA), (1, poB)):
                        # evict PSUM -> SBUF fast so the po slot frees, then
                        # normalize out of SBUF (PSUM-direct custom-DVE reads
                        # diverge on HW)
                        pvt = wk.tile([D, 512], F32, tag="pvt", bufs=4,
                                      name="pvt")
                        nc.vector.tensor_copy(pvt[:], po[0:D, :])
                        rs = wk.tile([1, 512], F32, tag="rs", bufs=4, name="rs")
                        nc.vector.tensor_copy(rs[:], po[D:D + 1, :])
                        rec = wk.tile([1, 512], F32, tag="rec", bufs=4,
                                      name="rec")
                        nc.vector.reciprocal_approx_fast(rec[:], rs[:])
                        rb = wk.tile([D, 512], F32, tag="rb", bufs=4, name="rb")
                        nc.gpsimd.partition_broadcast(rb[:], rec[:])
                        nc.vector.tensor_mul(
                            outT[p][j * D:(j + 1) * D, cs], pvt[:], rb[:])
                        drain(filler, 2)
                    if p == NP - 1 and n == 0:
                        # out rows 0..511 depend only on outT s-bank 0 —
                        # that half of the final projection (both e'-banks)
                        # becomes the PE filler for pair 7's bank-1 attention
                        filler = chain(
                            gen_outproj(0, wot_all[0], range(SM // 2)),
                            gen_outproj(1, wot_all[1], range(SM // 2)))

            # ---- phase 3: remaining final projection (s-bank 1) ----
            drain_all(filler)
            drain_all(gen_outproj(0, wot_all[0], range(SM // 2, SM)))
            drain_all(gen_outproj(1, wot_all[1], range(SM // 2, SM)))

    nc.finalize()
    return nc


_NC = None


def _get_nc():
    global _NC
    if _NC is None:
        _NC = build_nc()
    return _NC


def _prep_weights(W_qkv, b_qkv):
    # reference column order is (h, d, qkv) with qkv innermost
    W = np.asarray(W_qkv, dtype=np.float32).reshape(E, H, D, 3)
    b = np.asarray(b_qkv, dtype=np.float32).reshape(H, D, 3)
    Wq = W[..., 0].reshape(E, E)
    Wk = W[..., 1].reshape(E, E)
    Wv = W[..., 2].reshape(E, E)
    bq = b[..., 0].reshape(E)
    bk = b[..., 1].reshape(E)
    bv = b[..., 2].reshape(E)
    wqk = np.empty((E, 2 * E), dtype=np.float32)
    bqk = np.empty(2 * E, dtype=np.float32)
    for p in range(NP):
        wqk[:, p * 256:p * 256 + P] = Wq[:, p * P:(p + 1) * P]
        wqk[:, p * 256 + P:(p + 1) * 256] = Wk[:, p * P:(p + 1) * P]
        bqk[p * 256:p * 256 + P] = bq[p * P:(p + 1) * P]
        bqk[p * 256 + P:(p + 1) * 256] = bk[p * P:(p + 1) * P]
    return wqk, np.ascontiguousarray(Wv), bqk, bv


def kernel(x, W_qkv, b_qkv, W_out, b_out, _trace=False, _tmpdir=None):
    bf = ml_dtypes.bfloat16
    x = np.asarray(x, dtype=np.float32).astype(bf)
    wqk, wv, bqk, bv = _prep_weights(W_qkv, b_qkv)
    wqk = wqk.astype(bf)
    wv = wv.astype(bf)
    wout = np.ascontiguousarray(np.asarray(W_out, dtype=np.float32).astype(bf))
    bout = np.ascontiguousarray(np.asarray(b_out, dtype=np.float32))
    nc = _get_nc()
    in_maps = [
        {"xt": np.ascontiguousarray(x[i].T), "wqk": wqk, "wv": wv,
         "bqk": bqk, "bv": bv, "wout": wout, "bout": bout}
        for i in range(x.shape[0])
    ]
    res = run_bass_kernel_spmd(
        nc, in_maps, core_ids=list(range(x.shape[0])),
        trace=_trace, tmpdir=_tmpdir)
    outp = np.stack([rr["out"] for rr in res.results], axis=0)
    kernel.last_result = res
    return outp
